# revision 1
# baseline (speedup 1.0000x reference)
"""Multi-head causal self-attention (V=Q variant) on 8 Trainium2 cores.

Sharding: batch (2) x head-group (4 groups of 4 heads). Each core computes
full-sequence attention for its 4 heads of one batch element, plus its slice
of the output projection; the host sums the 4 partial projections per batch
and adds b0.

Per core: xT [1024, 2048], Wq_s/Wk_s [1024, 256], W0_s [256, 1024].

Scores are computed transposed (S^T[kv, q]) so the softmax denominator falls
out of the AV matmul via a ones-column appended to V (V aliases Q in this
module -- the reference replicates that bug). The 1/sqrt(DK) scale is folded
into Wk/bk on the host. All matmul operands are bf16 (1 cycle/row on the PE);
PSUM accumulation stays fp32.

The two heads of a pair share one [128,1024] score PSUM tile so a single ACT
exp instruction covers both (halves ACT instruction count). V' tiles are
produced by DMA-engine transposes (XBAR) straight from the bf16 QT tiles,
keeping the PE out of the transpose path and DVE out of the evictions. ACT
(exp) is the attention-phase rate limiter, so program order interleaves
projection sub-sweeps and output-projection blocks into the PE's exp-wait
gaps via a drip queue. Causal diagonal tiles are narrowed to their valid
q-range (left-trimmed).
"""

import ml_dtypes
import numpy as np

import concourse.bacc as bacc
import concourse.mybir as mybir
from concourse.tile import TileContext

P = 128
S = 2048  # sequence length
D = 1024  # model dim
HD = 256  # head-group width (4 heads x 64)
DK = 64
NQ = 4  # q chunks of 512
NKV = 16  # kv chunks of 128
NKD = 8  # D chunks of 128
F32 = mybir.dt.float32
BF16 = mybir.dt.bfloat16
EXP = mybir.ActivationFunctionType.Exp

_CACHED_NC = None


def build_nc():
    nc = bacc.Bacc("TRN2", target_bir_lowering=False, debug=False, num_devices=8)
    xT = nc.declare_dram_parameter("xT", [D, S], BF16, isOutput=False)
    Wq = nc.declare_dram_parameter("Wq", [D, HD], BF16, isOutput=False)
    Wk = nc.declare_dram_parameter("Wk", [D, HD], BF16, isOutput=False)
    bqt = nc.declare_dram_parameter("bqt", [P, 2], F32, isOutput=False)
    bkt = nc.declare_dram_parameter("bkt", [P, 2], F32, isOutput=False)
    W0 = nc.declare_dram_parameter("W0", [HD, D], BF16, isOutput=False)
    out = nc.declare_dram_parameter("out", [S, D], BF16, isOutput=True)

    with TileContext(nc) as tc:
        with (
            tc.tile_pool(name="const", bufs=1) as const,
            tc.tile_pool(name="xt", bufs=16) as xtp,
            tc.tile_pool(name="wqk", bufs=1) as wp,
            tc.tile_pool(name="vp", bufs=32) as vpool,
            tc.tile_pool(name="pt", bufs=6) as ptp,
            tc.tile_pool(name="dp", bufs=6) as dpool,
            tc.tile_pool(name="ost", bufs=5) as ostp,
            tc.tile_pool(name="mm", bufs=2, space="PSUM") as mmp,
            tc.tile_pool(name="sps", bufs=2, space="PSUM") as spsum,
            tc.tile_pool(name="aps", bufs=2, space="PSUM") as apsum,
        ):
            ones_col = const.tile([P, 1], BF16)
            nc.gpsimd.memset(ones_col[:], 1.0)
            identity = const.tile([P, P], BF16)
            nc.gpsimd.memset(identity[:], 0.0)
            nc.gpsimd.affine_select(
                out=identity[:],
                in_=identity[:],
                compare_op=mybir.AluOpType.not_equal,
                fill=1.0,
                base=0,
                pattern=[[-1, P]],
                channel_multiplier=1,
            )
            # triangular mask [128,128] bf16: keep (1.0) where q >= kv
            tri = const.tile([P, P], BF16, name="tri")
            nc.gpsimd.memset(tri[:], 1.0)
            nc.gpsimd.affine_select(
                out=tri[:],
                in_=tri[:],
                compare_op=mybir.AluOpType.is_ge,
                fill=0.0,
                base=0,
                pattern=[[1, P]],
                channel_multiplier=-1,
            )
            # ACT exp-table warmup while DMAs run
            warm = const.tile([P, 8], F32, name="warm")
            nc.gpsimd.memset(warm[:], 0.0)
            nc.scalar.activation(out=warm[:], in_=warm[:], func=EXP)

            bq_sb = const.tile([P, 2], F32)
            bk_sb = const.tile([P, 2], F32)
            w0_sb = [const.tile([P, D], BF16, name=f"w0_{kc}") for kc in range(2)]
            # QT/KT as [mi][ni] tiles of [128, 512] for fine-grained deps
            QT = [
                [const.tile([P, 512], BF16, name=f"qt{mi}_{ni}") for ni in range(NQ)]
                for mi in range(2)
            ]
            KT = [
                [const.tile([P, 512], BF16, name=f"kt{mi}_{ni}") for ni in range(NQ)]
                for mi in range(2)
            ]
            # normalized attention (transposed), per q-chunk and head-pair
            attn = [
                [const.tile([P, 512], BF16, name=f"attn{j}_{p}") for p in range(2)]
                for j in range(4)
            ]

            # first k-slice of each weight as its own tile/DMA so the first
            # projection sub-sweep starts as soon as ~130KB has landed
            wq_t0 = wp.tile([P, HD], BF16, name="wq0")
            wk_t0 = wp.tile([P, HD], BF16, name="wk0")
            wq_r = wp.tile([P, NKD - 1, HD], BF16, name="wqr")
            wk_r = wp.tile([P, NKD - 1, HD], BF16, name="wkr")
            wq_view = Wq.rearrange("(k p) c -> p k c", p=P)
            wk_view = Wk.rearrange("(k p) c -> p k c", p=P)
            nc.sync.dma_start(out=wq_t0[:], in_=wq_view[:, 0, :])
            xh = [
                [xtp.tile([P, 1024], BF16, name="xtile") for _ in range(2)]
                for _ in range(NKD)
            ]
            nc.sync.dma_start(out=xh[0][0][:], in_=xT[0:P, 0:1024])
            nc.sync.dma_start(out=wk_t0[:], in_=wk_view[:, 0, :])
            nc.sync.dma_start(out=bq_sb[:], in_=bqt[:, :])
            nc.sync.dma_start(out=bk_sb[:], in_=bkt[:, :])
            nc.sync.dma_start(out=wq_r[:], in_=wq_view[:, 1:NKD, :])
            nc.sync.dma_start(out=wk_r[:], in_=wk_view[:, 1:NKD, :])
            wq_t = [wq_t0[:]] + [wq_r[:, k - 1, :] for k in range(1, NKD)]
            wk_t = [wk_t0[:]] + [wk_r[:, k - 1, :] for k in range(1, NKD)]
            # x half-0 chunks first, then half-1, then W0: the DMA path
            # drains in issue order, which staggers arrivals naturally
            for h in range(2):
                for k in range(NKD):
                    if h == 0 and k == 0:
                        continue
                    nc.sync.dma_start(
                        out=xh[k][h][:],
                        in_=xT[k * P : (k + 1) * P, h * 1024 : (h + 1) * 1024],
                    )
            for kc in range(2):
                nc.sync.dma_start(out=w0_sb[kc][:], in_=W0[kc * P : (kc + 1) * P, :])

            def sweep_items(ni, mi):
                """Projection sub-sweep as a list of emit-thunks (per-k)."""
                half, col = divmod(ni, 2)
                pss = [mmp.tile([P, 512], F32, name="ps") for _ in range(2)]

                def mk(k):
                    def go():
                        for ps, wt in zip(pss, (wq_t, wk_t)):
                            nc.tensor.matmul(
                                ps[:],
                                lhsT=wt[k][:, mi * P : (mi + 1) * P],
                                rhs=xh[k][half][:, col * 512 : (col + 1) * 512],
                                start=(k == 0),
                                stop=(k == NKD - 1),
                            )
                    return go

                def evict():
                    # in the DMA-paced startup ACT is idle: the K eviction of
                    # the first two sweeps runs there so QT/KT land sooner
                    early = ni < 2 and mi == 0
                    for n_, (ps, bias, dstT) in enumerate(
                        zip(pss, (bq_sb, bk_sb), (QT, KT))
                    ):
                        if early and n_ == 1:
                            nc.scalar.add(
                                dstT[mi][ni][:, :], ps[:], bias[:, mi : mi + 1]
                            )
                        else:
                            nc.vector.tensor_scalar_add(
                                dstT[mi][ni][:, :], ps[:], bias[:, mi : mi + 1]
                            )

                return [mk(k) for k in range(NKD)] + [evict]

            vp = {}

            def emit_transposes(pair, i_lo, i_hi):
                # V' tiles [128, 132] via XBAR dma transpose from QT:
                # A data 0:64, A one 64, B data 66:130, B one 130.
                for i in range(i_lo, i_hi):
                    if (pair, i) in vp:
                        continue
                    src = QT[pair][i // 4][:, (i % 4) * P : (i % 4 + 1) * P]
                    # B data sits at col 80: XBAR writes require the out
                    # offset to be 32-byte aligned (16 bf16 cols)
                    vt = vpool.tile([P, 148], BF16, name="vt")
                    nc.sync.dma_start_transpose(out=vt[:, 0:64], in_=src[0:64, :])
                    nc.sync.dma_start_transpose(out=vt[:, 80:144], in_=src[64:128, :])
                    nc.gpsimd.tensor_copy(vt[:, 64:65], ones_col[:])
                    nc.gpsimd.tensor_copy(vt[:, 144:145], ones_col[:])
                    vp[(pair, i)] = vt

            def emit_transposes_pe(pair, i_lo, i_hi):
                # startup variant: PE transposes (PE is idle while DMAs land,
                # and the DMA/HWDGE path is saturated with input loads)
                # shares the "aps" slots: dead before the first atB allocation
                tp = apsum.tile([P, 512], BF16, name="aps")
                for i in range(i_lo, i_hi):
                    c = (i - i_lo) * P
                    nc.tensor.transpose(
                        tp[:, c : c + P],
                        QT[pair][i // 4][:, (i % 4) * P : (i % 4 + 1) * P],
                        identity[:],
                    )
                    vt = vpool.tile([P, 148], BF16, name="vt")
                    nc.vector.tensor_copy(vt[:, 0:64], tp[:, c : c + 64])
                    nc.vector.tensor_copy(vt[:, 80:144], tp[:, c + 64 : c + P])
                    nc.gpsimd.tensor_copy(vt[:, 64:65], ones_col[:])
                    nc.gpsimd.tensor_copy(vt[:, 144:145], ones_col[:])
                    vp[(pair, i)] = vt

            bg = []  # drip queue of (cost, key, thunk)
            carry = [0.0]

            def drip(budget):
                # carry-based pacing: costs are ~213ns PE units; fractional
                # budget accumulates so fill work spreads over the whole
                # attention phase instead of draining the queue early
                carry[0] += budget
                while bg and carry[0] >= bg[0][0]:
                    cost, _key, thunk = bg.pop(0)
                    thunk()
                    carry[0] -= cost

            def ensure(key):
                # need-driven drain: emit every queued item with this key now
                # (deadline reached), regardless of queue position
                rest, hits = [], []
                for item in bg:
                    (hits if item[1] == key else rest).append(item)
                if hits:
                    bg[:] = rest
                    for _cost, _key, thunk in hits:
                        thunk()

            def emit_cblock_m(j, c, act_only=False):
                m = j * 4 + c
                ot = ostp.tile([P, D], BF16, name="ot")
                for n in range(2):
                    ps = mmp.tile([P, 512], F32, name="ps")
                    for kc in range(2):
                        nc.tensor.matmul(
                            ps[:],
                            lhsT=attn[j][kc][:, c * P : (c + 1) * P],
                            rhs=w0_sb[kc][:, n * 512 : (n + 1) * 512],
                            start=(kc == 0),
                            stop=(kc == 1),
                        )
                    osl = slice(n * 512, (n + 1) * 512)
                    # n=0 evicts on ACT to keep DVE clear for the normalize
                    # chain (which gates the next q-chunk's AV psum); in the
                    # endgame (act_only) ACT takes both halves since the DVE
                    # is the pacing engine there
                    if n == 0 or act_only:
                        nc.scalar.copy(ot[:, osl], ps[:])
                    else:
                        nc.vector.tensor_copy(ot[:, osl], ps[:])
                    if act_only:
                        # endgame: per-half DMA so the n=0 half transfers
                        # while the n=1 half is still evicting
                        nc.sync.dma_start(
                            out=out[m * P : (m + 1) * P, osl], in_=ot[:, osl]
                        )
                if not act_only:
                    nc.sync.dma_start(out=out[m * P : (m + 1) * P, :], in_=ot[:])

            def emit_pair(pair):
                steps = [(j, i) for j in range(NQ) for i in range(4 * j + 4)]
                ats = {}

                def emit_S(j, i):
                    off = max(0, i * P - j * 512)  # 0,128,256,384
                    w = 512 - off
                    kc = slice((i % 4) * P, (i % 4 + 1) * P)
                    sAB = spsum.tile([P, 1024], F32, name="sAB")
                    nc.tensor.matmul(
                        sAB[:, off:512],
                        lhsT=KT[pair][i // 4][0:64, kc],
                        rhs=QT[pair][j][0:64, off:512],
                    )
                    nc.tensor.matmul(
                        sAB[:, 512 + off : 1024],
                        lhsT=KT[pair][i // 4][64:128, kc],
                        rhs=QT[pair][j][64:128, off:512],
                    )
                    pAB = ptp.tile([P, 1024], BF16, name="pAB")
                    if w == 512:
                        nc.scalar.activation(out=pAB[:, :], in_=sAB[:, :], func=EXP)
                    else:
                        nc.scalar.activation(
                            out=pAB[:].rearrange("p (t c) -> p t c", t=2)[:, :, off:512],
                            in_=sAB[:].rearrange("p (t c) -> p t c", t=2)[:, :, off:512],
                            func=EXP,
                        )
                    if i >= 4 * j:  # diagonal tile: mask the leading block
                        nc.vector.tensor_mul(
                            pAB[:, off : off + P], pAB[:, off : off + P], tri[:]
                        )
                        nc.vector.tensor_mul(
                            pAB[:, 512 + off : 512 + off + P],
                            pAB[:, 512 + off : 512 + off + P],
                            tri[:],
                        )
                    return (j, i, pAB, off, w)

                def emit_AV(j, i, pAB, off, w):
                    if i == 0:
                        ats[j] = (
                            apsum.tile([P, 512], F32, name="aps"),
                            apsum.tile([P, 512], F32, name="aps"),
                        )
                    atA, atB = ats[j]
                    imax = 4 * j + 3
                    qsl = slice(off, 512)
                    vt = vp[(pair, i)]
                    last = i == imax

                    nc.tensor.matmul(
                        atA[0:65, qsl],
                        lhsT=vt[:, 0:65],
                        rhs=pAB[:, off:512],
                        start=(i == 0),
                        stop=last,
                    )
                    final = pair == 1 and j == NQ - 1
                    recA = None
                    if last and not final:
                        # head A's reciprocal runs while B's AV is on PE
                        recA = dpool.tile([1, 512], F32, name="rec")
                        nc.vector.reciprocal(recA[:], atA[64:65, :])
                    nc.tensor.matmul(
                        atB[0:65, qsl],
                        lhsT=vt[:, 80:145],
                        rhs=pAB[:, 512 + off : 1024],
                        start=(i == 0),
                        stop=last,
                    )
                    if last:
                        if final:
                            # final tile: the whole normalize chain runs at
                            # column-quarter granularity, each cblock emitted
                            # right after the quarter it consumes, so the
                            # first cblock starts ~1us after the last AV
                            rA = dpool.tile([1, 512], F32, name="rec")
                            rB = dpool.tile([1, 512], F32, name="rec")
                            bA = dpool.tile([64, 512], F32, name="rbc")
                            bB = dpool.tile([64, 512], F32, name="rbc")
                            for c in range(4):
                                cs = slice(c * P, (c + 1) * P)
                                nc.vector.reciprocal(rA[:, cs], atA[64:65, cs])
                                nc.vector.reciprocal(rB[:, cs], atB[64:65, cs])
                                nc.gpsimd.partition_broadcast(bA[0:64, cs], rA[0:1, cs])
                                nc.gpsimd.partition_broadcast(bB[0:64, cs], rB[0:1, cs])
                                nc.vector.tensor_mul(
                                    attn[j][pair][0:64, cs], atA[0:64, cs], bA[0:64, cs]
                                )
                                nc.vector.tensor_mul(
                                    attn[j][pair][64:128, cs], atB[0:64, cs], bB[0:64, cs]
                                )
                                emit_cblock_m(j, c, act_only=True)
                            return
                        # normalize attn = att_un / d (d = psum row 64), with
                        # the A and B chains pipelined across DVE and Pool and
                        # the muls in column halves so cblocks unblock early
                        recB = dpool.tile([1, 512], F32, name="rec")
                        nc.vector.reciprocal(recB[:], atB[64:65, :])
                        rbcA = dpool.tile([64, 512], F32, name="rbc")
                        nc.gpsimd.partition_broadcast(rbcA[0:64, :], recA[0:1, :])
                        rbcB = dpool.tile([64, 512], F32, name="rbc")
                        nc.gpsimd.partition_broadcast(rbcB[0:64, :], recB[0:1, :])
                        for h in range(2):
                            cs = slice(h * 256, (h + 1) * 256)
                            nc.vector.tensor_mul(
                                attn[j][pair][0:64, cs], atA[0:64, cs], rbcA[0:64, cs]
                            )
                            nc.vector.tensor_mul(
                                attn[j][pair][64:128, cs], atB[0:64, cs], rbcB[0:64, cs]
                            )
                        if pair == 1:  # output projection becomes available
                            for c in range(4):
                                bg.append(
                                    (4, ("cb", j, c), lambda j=j, c=c: emit_cblock_m(j, c))
                                )

                pend = []
                for j, i in steps:
                    if i == 0:
                        # deadline: this j's QT/KT sweep must be fully emitted
                        ensure(("sw", j, pair))
                    if (pair, i) not in vp:
                        emit_transposes(pair, i, i + 1)
                    pend.append(emit_S(j, i))
                    drip(3.5 if i <= 2 else 1.25)
                    # depth-2 software pipeline: AV(T) trails S(T) by two
                    # tiles, so the exp on ACT has two tile-periods of slack
                    if len(pend) > 3:
                        emit_AV(*pend.pop(0))
                for p_ in pend:
                    emit_AV(*p_)

            def t_item(pair, i):
                return (0, ("t", pair, i), lambda: emit_transposes(pair, i, i + 1))

            def sweep_bg(ni, mi):
                # per-k items are 2 matmuls of 512 cols (~427ns); evict is DVE
                return [
                    (2 if ix < NKD else 0, ("sw", ni, mi), it)
                    for ix, it in enumerate(sweep_items(ni, mi))
                ]

            # upfront: pair-0 ni=0 projection (DMA-paced) + first V transposes
            for it in sweep_items(0, 0):
                it()
            emit_transposes_pe(0, 0, 4)
            # bg order follows need-by and DMA-arrival order
            bg.extend(sweep_bg(1, 0))
            bg.extend(t_item(0, i) for i in range(4, 8))
            bg.extend(sweep_bg(2, 0))
            bg.extend(t_item(0, i) for i in range(8, 12))
            bg.extend(sweep_bg(3, 0))
            bg.extend(t_item(0, i) for i in range(12, 16))
            bg.extend(sweep_bg(0, 1))
            bg.extend(sweep_bg(1, 1))
            bg.extend(sweep_bg(2, 1))
            bg.extend(sweep_bg(3, 1))
            bg.extend(t_item(1, i) for i in range(0, 16))
            emit_pair(0)
            emit_pair(1)
            while bg:
                drip(8)

    nc.compile()
    return nc


def make_in_maps(pos_encode_toks, Wq, bq, Wk, bk, W0, b0):
    x = np.asarray(pos_encode_toks, dtype=np.float32)
    Wq = np.asarray(Wq, dtype=np.float32)
    bq = np.asarray(bq, dtype=np.float32)
    Wk = np.asarray(Wk, dtype=np.float32)
    bk = np.asarray(bk, dtype=np.float32)
    W0 = np.asarray(W0, dtype=np.float32)
    in_maps = []
    for core in range(8):
        b, g = divmod(core, 4)
        hs = slice(g * HD, (g + 1) * HD)
        scale = np.float32(1.0 / np.sqrt(DK))
        in_maps.append(
            {
                "xT": np.ascontiguousarray(x[b].T).astype(ml_dtypes.bfloat16),
                "Wq": np.ascontiguousarray(Wq[:, hs]).astype(ml_dtypes.bfloat16),
                "Wk": np.ascontiguousarray(Wk[:, hs] * scale).astype(ml_dtypes.bfloat16),
                "bqt": np.ascontiguousarray(bq[hs].reshape(2, P).T),
                "bkt": np.ascontiguousarray((bk[hs] * scale).reshape(2, P).T),
                "W0": np.ascontiguousarray(W0[hs, :]).astype(ml_dtypes.bfloat16),
            }
        )
    return in_maps


def assemble(results, b0):
    out = np.zeros((2, S, D), dtype=np.float32)
    for core in range(8):
        b = core // 4
        out[b] += results[core]["out"].astype(np.float32)
    out += np.asarray(b0, dtype=np.float32)
    return out


def kernel(pos_encode_toks, Wq, bq, Wk, bk, W0, b0):
    from concourse.bass_utils import run_bass_kernel_spmd

    global _CACHED_NC
    if _CACHED_NC is None:
        _CACHED_NC = build_nc()
    in_maps = make_in_maps(pos_encode_toks, Wq, bq, Wk, bk, W0, b0)
    res = run_bass_kernel_spmd(_CACHED_NC, in_maps, core_ids=list(range(8)))
    return assemble(res.results, b0)



# revision 48
# speedup vs baseline: 1.0881x; 1.0881x over previous
"""Multi-head causal self-attention (V=Q variant) on 8 Trainium2 cores.

Sharding: batch (2) x head-group (4 groups of 4 heads). Each core computes
full-sequence attention for its 4 heads of one batch element, plus its slice
of the output projection; the host sums the 4 partial projections per batch
and adds b0.

Per core: xT [1024, 2048], Wq_s/Wk_s [1024, 256], W0_s [256, 1024].

Scores are computed transposed (S^T[kv, q]) so the softmax denominator falls
out of the AV matmul via a ones-column appended to V (V aliases Q in this
module -- the reference replicates that bug). The 1/sqrt(DK) scale is folded
into Wk/bk on the host. All matmul operands are bf16 (1 cycle/row on the PE);
PSUM accumulation stays fp32.

The two heads of a pair share one [128,1024] score PSUM tile so a single ACT
exp instruction covers both (halves ACT instruction count). V' tiles are
produced by DMA-engine transposes (XBAR) straight from the bf16 QT tiles,
keeping the PE out of the transpose path and DVE out of the evictions. ACT
(exp) is the attention-phase rate limiter, so program order interleaves
projection sub-sweeps and output-projection blocks into the PE's exp-wait
gaps via a drip queue. Causal diagonal tiles are narrowed to their valid
q-range (left-trimmed).
"""

import ml_dtypes
import numpy as np

import concourse.bacc as bacc
import concourse.mybir as mybir
from concourse.tile import TileContext

P = 128
S = 2048  # sequence length
D = 1024  # model dim
HD = 256  # head-group width (4 heads x 64)
DK = 64
NQ = 4  # q chunks of 512
NKV = 16  # kv chunks of 128
NKD = 8  # D chunks of 128
F32 = mybir.dt.float32
BF16 = mybir.dt.bfloat16
FP8 = mybir.dt.float8e4
DR = mybir.MatmulPerfMode.DoubleRow
EXP = mybir.ActivationFunctionType.Exp
SCALE = 1.0 / np.sqrt(DK)  # applied inside the exp activation

_CACHED_NC = None


def build_nc():
    nc = bacc.Bacc("TRN2", target_bir_lowering=False, debug=False, num_devices=8)
    xT = nc.declare_dram_parameter("xT", [D, S], FP8, isOutput=False)
    xTl = nc.declare_dram_parameter("xTl", [D, S], FP8, isOutput=False)
    Wq = nc.declare_dram_parameter("Wq", [D, HD], FP8, isOutput=False)
    Wk = nc.declare_dram_parameter("Wk", [D, HD], FP8, isOutput=False)
    Wql = nc.declare_dram_parameter("Wql", [D, HD], FP8, isOutput=False)
    Wkl = nc.declare_dram_parameter("Wkl", [D, HD], FP8, isOutput=False)
    bqt = nc.declare_dram_parameter("bqt", [P, 2], F32, isOutput=False)
    bkt = nc.declare_dram_parameter("bkt", [P, 2], F32, isOutput=False)
    W0 = nc.declare_dram_parameter("W0", [HD, D], BF16, isOutput=False)
    out = nc.declare_dram_parameter("out", [S, D], BF16, isOutput=True)

    with TileContext(nc) as tc:
        with (
            tc.tile_pool(name="const", bufs=1) as const,
            tc.tile_pool(name="xt", bufs=1) as xtp,
            tc.tile_pool(name="wqk", bufs=1) as wp,
            tc.tile_pool(name="vp", bufs=32) as vpool,
            tc.tile_pool(name="pt", bufs=34) as ptp,
            tc.tile_pool(name="dp", bufs=8) as dpool,
            tc.tile_pool(name="an", bufs=3) as anp,
            tc.tile_pool(name="ost", bufs=5) as ostp,
            tc.tile_pool(name="mm", bufs=4, space="PSUM") as mmp,
            tc.tile_pool(name="sps", bufs=2, space="PSUM") as spsum,
        ):
            ones_col = const.tile([P, 1], BF16)
            nc.gpsimd.memset(ones_col[:], 1.0)
            identity = const.tile([P, P], BF16)
            nc.gpsimd.memset(identity[:], 0.0)
            nc.gpsimd.affine_select(
                out=identity[:],
                in_=identity[:],
                compare_op=mybir.AluOpType.not_equal,
                fill=1.0,
                base=0,
                pattern=[[-1, P]],
                channel_multiplier=1,
            )
            # triangular mask [128,128] bf16: keep (1.0) where q >= kv
            tri = const.tile([P, P], BF16, name="tri")
            nc.gpsimd.memset(tri[:], 1.0)
            nc.gpsimd.affine_select(
                out=tri[:],
                in_=tri[:],
                compare_op=mybir.AluOpType.is_ge,
                fill=0.0,
                base=0,
                pattern=[[1, P]],
                channel_multiplier=-1,
            )
            # ACT exp-table warmup while DMAs run
            warm = const.tile([P, 8], F32, name="warm")
            nc.gpsimd.memset(warm[:], 0.0)
            nc.scalar.activation(out=warm[:], in_=warm[:], func=EXP)
            # PE p-state warmup: ~3us of dummy transposes on the identity
            # tile (no DMA dependency) so the PE is at full clock when the
            # first projection data lands
            pewarm = mmp.tile([P, P], BF16, name="ps")
            for _ in range(60):
                nc.tensor.transpose(pewarm[:], identity[:], identity[:])

            bq_sb = const.tile([P, 2], F32)
            bk_sb = const.tile([P, 2], F32)
            w0_sb = [const.tile([P, D], BF16, name=f"w0_{kc}") for kc in range(2)]
            # QT/KT as [mi][ni] tiles of [128, 512] for fine-grained deps
            QT = [
                [const.tile([P, 512], BF16, name=f"qt{mi}_{ni}") for ni in range(NQ)]
                for mi in range(2)
            ]
            KT = [
                [const.tile([P, 512], BF16, name=f"kt{mi}_{ni}") for ni in range(NQ)]
                for mi in range(2)
            ]
            # normalized attention (transposed), per q-chunk and head-pair
            attn = [
                [const.tile([P, 512], BF16, name=f"attn{j}_{p}") for p in range(2)]
                for j in range(4)
            ]

            # first k-pair of each weight as its own tile/DMA so the first
            # projection sub-sweep starts as soon as ~130KB has landed
            # weights split per mi (head-pair column half) so the first-exp
            # critical path only waits on the mi=0 halves
            wq_m = [wp.tile([P, NKD, P], FP8, name=f"wq{m}") for m in range(2)]
            wk_m = [wp.tile([P, NKD, P], FP8, name=f"wk{m}") for m in range(2)]
            wql_m = [wp.tile([P, NKD, P], FP8, name=f"wql{m}") for m in range(2)]
            wkl_m = [wp.tile([P, NKD, P], FP8, name=f"wkl{m}") for m in range(2)]
            wq_view = Wq.rearrange("(k p) c -> p k c", p=P)
            wk_view = Wk.rearrange("(k p) c -> p k c", p=P)
            wql_view = Wql.rearrange("(k p) c -> p k c", p=P)
            wkl_view = Wkl.rearrange("(k p) c -> p k c", p=P)
            # DMA order = first-exp critical path: mi=0 hi weights, the first
            # 512-col sliver of x (all 8 k-tiles in one batched DMA), mi=0 lo
            # weights; mi=1 weights and remaining x col-blocks follow.
            xall = xtp.tile([P, NKD, S], FP8, name="xall")
            xlo = xtp.tile([P, NKD, S], FP8, name="xlo")
            xT_view = xT.rearrange("(k p) s -> p k s", p=P)
            xTl_view = xTl.rearrange("(k p) s -> p k s", p=P)
            ms0 = slice(0, P)
            ms1 = slice(P, HD)
            nc.sync.dma_start(out=wq_m[0][:], in_=wq_view[:, :, ms0])
            nc.sync.dma_start(out=wk_m[0][:], in_=wk_view[:, :, ms0])
            nc.sync.dma_start(out=xall[:, :, 0:512], in_=xT_view[:, :, 0:512])
            nc.sync.dma_start(out=xlo[:, :, 0:512], in_=xTl_view[:, :, 0:512])
            nc.sync.dma_start(out=wql_m[0][:], in_=wql_view[:, :, ms0])
            nc.sync.dma_start(out=wkl_m[0][:], in_=wkl_view[:, :, ms0])
            nc.sync.dma_start(out=bq_sb[:], in_=bqt[:, :])
            nc.sync.dma_start(out=bk_sb[:], in_=bkt[:, :])
            nc.sync.dma_start(out=xall[:, :, 512:1024], in_=xT_view[:, :, 512:1024])
            nc.sync.dma_start(out=xlo[:, :, 512:1024], in_=xTl_view[:, :, 512:1024])
            nc.sync.dma_start(out=wq_m[1][:], in_=wq_view[:, :, ms1])
            nc.sync.dma_start(out=wk_m[1][:], in_=wk_view[:, :, ms1])
            nc.sync.dma_start(out=wql_m[1][:], in_=wql_view[:, :, ms1])
            nc.sync.dma_start(out=wkl_m[1][:], in_=wkl_view[:, :, ms1])
            nc.sync.dma_start(out=xall[:, :, 1024:2048], in_=xT_view[:, :, 1024:2048])
            nc.sync.dma_start(out=xlo[:, :, 1024:2048], in_=xTl_view[:, :, 1024:2048])
            for kc in range(2):
                nc.sync.dma_start(out=w0_sb[kc][:], in_=W0[kc * P : (kc + 1) * P, :])

            def wpair(wm, kp, mi):
                return wm[mi][:, 2 * kp : 2 * kp + 2, :]

            def sweep_items(ni, mi, which):
                """One projection (q or k) sub-sweep as emit-thunks.

                fp8 DoubleRow with residual compensation, all terms scaled
                x16 so they share ONE psum group (DVE reads a single PSUM
                stream): (16 W_hi).x_hi + (16 W_lo).x_hi + W_hi.(16 x_lo).
                The eviction computes psum/16 + bias in one tensor_scalar.
                """
                half, col = divmod(ni, 2)
                c0 = half * 1024 + col * 512
                ps = mmp.tile([P, 512], F32, name="ps")
                wm = wq_m if which == 0 else wk_m
                lm = wql_m if which == 0 else wkl_m
                bias = bq_sb if which == 0 else bk_sb
                dstT = QT if which == 0 else KT

                def mk(kp):
                    def go():
                        nc.tensor.matmul(
                            ps[:],
                            lhsT=wpair(wm, kp, mi),
                            rhs=xall[:, 2 * kp : 2 * kp + 2, c0 : c0 + 512],
                            start=(kp == 0),
                            stop=False,
                            perf_mode=DR,
                        )
                        nc.tensor.matmul(
                            ps[:],
                            lhsT=wpair(wm, kp, mi),
                            rhs=xlo[:, 2 * kp : 2 * kp + 2, c0 : c0 + 512],
                            start=False,
                            stop=False,
                            perf_mode=DR,
                        )
                        nc.tensor.matmul(
                            ps[:],
                            lhsT=wpair(lm, kp, mi),
                            rhs=xall[:, 2 * kp : 2 * kp + 2, c0 : c0 + 512],
                            start=False,
                            stop=(kp == NKD // 2 - 1),
                            perf_mode=DR,
                        )
                    return go

                def evict():
                    nc.vector.tensor_scalar(
                        out=dstT[mi][ni][:, :],
                        in0=ps[:],
                        scalar1=1.0 / 16.0,
                        scalar2=bias[:, mi : mi + 1],
                        op0=mybir.AluOpType.mult,
                        op1=mybir.AluOpType.add,
                    )

                return [mk(kp) for kp in range(NKD // 2)] + [evict]

            vp = {}

            def emit_transposes(pair, i_lo, i_hi):
                # V' tiles [128, 132] via XBAR dma transpose from QT:
                # A data 0:64, A one 64, B data 66:130, B one 130.
                for i in range(i_lo, i_hi):
                    if (pair, i) in vp:
                        continue
                    src = QT[pair][i // 4][:, (i % 4) * P : (i % 4 + 1) * P]
                    # B data sits at col 80: XBAR writes require the out
                    # offset to be 32-byte aligned (16 bf16 cols)
                    vt = vpool.tile([P, 148], BF16, name="vt")
                    nc.sync.dma_start_transpose(out=vt[:, 0:64], in_=src[0:64, :])
                    nc.sync.dma_start_transpose(out=vt[:, 80:144], in_=src[64:128, :])
                    nc.gpsimd.tensor_copy(vt[:, 64:65], ones_col[:])
                    nc.gpsimd.tensor_copy(vt[:, 144:145], ones_col[:])
                    vp[(pair, i)] = vt

            def emit_transposes_pe(pair, i_lo, i_hi):
                # startup variant: PE transposes (PE is idle while DMAs land,
                # and the DMA/HWDGE path is saturated with input loads)
                tp = mmp.tile([P, 512], BF16, name="ps")
                for i in range(i_lo, i_hi):
                    c = (i - i_lo) * P
                    nc.tensor.transpose(
                        tp[:, c : c + P],
                        QT[pair][i // 4][:, (i % 4) * P : (i % 4 + 1) * P],
                        identity[:],
                    )
                    vt = vpool.tile([P, 148], BF16, name="vt")
                    nc.vector.tensor_copy(vt[:, 0:64], tp[:, c : c + 64])
                    nc.vector.tensor_copy(vt[:, 80:144], tp[:, c + 64 : c + P])
                    nc.gpsimd.tensor_copy(vt[:, 64:65], ones_col[:])
                    nc.gpsimd.tensor_copy(vt[:, 144:145], ones_col[:])
                    vp[(pair, i)] = vt

            bg = []  # drip queue of (cost, key, thunk)
            carry = [0.0]

            def drip(budget):
                # carry-based pacing: costs are ~213ns PE units; fractional
                # budget accumulates so fill work spreads over the whole
                # attention phase instead of draining the queue early
                carry[0] += budget
                while bg and carry[0] >= bg[0][0]:
                    cost, _key, thunk = bg.pop(0)
                    thunk()
                    carry[0] -= cost

            def ensure(key):
                # need-driven drain: emit every queued item with this key now
                # (deadline reached), regardless of queue position
                rest, hits = [], []
                for item in bg:
                    (hits if item[1] == key else rest).append(item)
                if hits:
                    bg[:] = rest
                    for _cost, _key, thunk in hits:
                        thunk()

            cb_state = {}

            def emit_cblock_half(j, c, n, act_only=False):
                m = j * 4 + c
                if n == 0:
                    cb_state[(j, c)] = ostp.tile([P, D], BF16, name="ot")
                ot = cb_state[(j, c)]
                ps = mmp.tile([P, 512], F32, name="ps")
                for kc in range(2):
                    nc.tensor.matmul(
                        ps[:],
                        lhsT=attn[j][kc][:, c * P : (c + 1) * P],
                        rhs=w0_sb[kc][:, n * 512 : (n + 1) * 512],
                        start=(kc == 0),
                        stop=(kc == 1),
                    )
                osl = slice(n * 512, (n + 1) * 512)
                # steady state: DVE evicts (ACT is saturated with exp);
                # endgame (act_only): ACT evicts -- its exp stream is done
                # and the DVE queue is the endgame critical path
                if act_only:
                    nc.scalar.copy(ot[:, osl], ps[:])
                else:
                    nc.vector.tensor_copy(ot[:, osl], ps[:])
                if act_only:
                    # endgame: per-half DMA so the n=0 half transfers
                    # while the n=1 half is still evicting
                    nc.sync.dma_start(out=out[m * P : (m + 1) * P, osl], in_=ot[:, osl])
                elif n == 1:
                    nc.sync.dma_start(out=out[m * P : (m + 1) * P, :], in_=ot[:])

            def emit_cblock_m(j, c, act_only=False):
                for n in range(2):
                    emit_cblock_half(j, c, n, act_only)

            def pair_gen(pair):
                """Generator: one attention step per yield, so the two
                head-pairs' streams can interleave and fill each other's
                dependency stalls."""
                steps = [(j, i) for j in range(NQ) for i in range(4 * j + 4)]
                pabs = {}
                asbs = {}

                def emit_S(j, i):
                    off = max(0, i * P - j * 512)  # 0,128,256,384
                    w = 512 - off
                    kc = slice((i % 4) * P, (i % 4 + 1) * P)
                    sAB = spsum.tile([P, 1024], F32, name="sAB")
                    nc.tensor.matmul(
                        sAB[:, off:512],
                        lhsT=KT[pair][i // 4][0:64, kc],
                        rhs=QT[pair][j][0:64, off:512],
                    )
                    nc.tensor.matmul(
                        sAB[:, 512 + off : 1024],
                        lhsT=KT[pair][i // 4][64:128, kc],
                        rhs=QT[pair][j][64:128, off:512],
                    )
                    pAB = ptp.tile([P, 1024], BF16, name="pAB")
                    if w == 512:
                        nc.scalar.activation(
                            out=pAB[:, :], in_=sAB[:, :], func=EXP, scale=SCALE
                        )
                    else:
                        nc.scalar.activation(
                            out=pAB[:].rearrange("p (t c) -> p t c", t=2)[:, :, off:512],
                            in_=sAB[:].rearrange("p (t c) -> p t c", t=2)[:, :, off:512],
                            func=EXP,
                            scale=SCALE,
                        )
                    if i >= 4 * j:  # diagonal tile: mask the leading block
                        nc.vector.tensor_mul(
                            pAB[:, off : off + P], pAB[:, off : off + P], tri[:]
                        )
                        nc.vector.tensor_mul(
                            pAB[:, 512 + off : 512 + off + P],
                            pAB[:, 512 + off : 512 + off + P],
                            tri[:],
                        )
                    pabs[(j, i)] = pAB

                def av_burst(j, qb):
                    # transposed AV: out = attn-normal [128 q-part, 65] for
                    # one 128-q block -- the PE streams only 65 columns per
                    # kv tile (vs 512 in the [65, q] orientation) and the
                    # softmax denominator lands per-partition (column 64).
                    # One accumulation group per PSUM bank (full-bank slots;
                    # the PE tracks a single active accum range per bank).
                    for hh in range(2):
                        if qb == 0:
                            asbs[(j, hh)] = dpool.tile([P, 4, 65], F32, name="asb")
                        av = mmp.tile([P, 512], F32, name="ps")
                        for t in range(4 * j + qb + 1):
                            vt = vp[(pair, t)]
                            vsl = vt[:, 0:65] if hh == 0 else vt[:, 80:145]
                            nc.tensor.matmul(
                                av[:, 0:65],
                                lhsT=pabs[(j, t)][
                                    :, hh * 512 + qb * P : hh * 512 + (qb + 1) * P
                                ],
                                rhs=vsl,
                                start=(t == 0),
                                stop=(t == 4 * j + qb),
                            )
                        # prompt psum->sbuf eviction frees the bank slot
                        nc.vector.tensor_copy(asbs[(j, hh)][:, qb, :], av[:, 0:65])

                def finalize(j):
                    # per-qb Pool normalize_recip divides the 64 dk cols by
                    # col 64 (the ones-column row sum), writing bf16
                    # attn-normal; PE transposes assemble the attn^T tile
                    # the out-projection consumes.
                    an = anp.tile([P, 4, 2, 64], BF16, name="an")
                    tpt = mmp.tile([P, 4, P], BF16, name="ps")
                    for qb in range(4):
                        for hh in range(2):
                            nc.gpsimd.normalize_recip(
                                an[:, qb, hh, :],
                                asbs[(j, hh)][:, qb, 0:64],
                                asbs[(j, hh)][:, qb, 64:65],
                            )
                        # transpose [128 q, 2x64 hd] -> [128 hd, 128 q] on PE
                        nc.tensor.transpose(
                            tpt[:, qb, :], an[:, qb, :, :], identity[:]
                        )
                    nc.vector.tensor_copy(attn[j][pair][:, :], tpt[:, :, :])
                    if pair == 1:  # output projection becomes available
                        for c in range(4):
                            for n in range(2):
                                bg.append(
                                    (
                                        2,
                                        ("cb", j, c),
                                        lambda j=j, c=c, n=n: emit_cblock_half(j, c, n),
                                    )
                                )

                def final_tail(j, qb):
                    # endgame (pair 1, last j): per-qb chain burst ->
                    # normalize -> transpose -> attn evict -> cblock, so the
                    # post-exp drain is pipelined across engines per q-block
                    av_burst(j, qb)
                    if qb == 0:
                        fin["an"] = anp.tile([P, 4, 2, 64], BF16, name="an")
                    an = fin["an"]
                    for hh in range(2):
                        nc.gpsimd.normalize_recip(
                            an[:, qb, hh, :],
                            asbs[(j, hh)][:, qb, 0:64],
                            asbs[(j, hh)][:, qb, 64:65],
                        )
                    # per-qb transpose psum: its pool slot must not persist
                    # across later burst allocations (4-slot rotation)
                    tpt = mmp.tile([P, P], BF16, name="ps")
                    nc.tensor.transpose(tpt[:, :], an[:, qb, :, :], identity[:])
                    nc.vector.tensor_copy(
                        attn[j][pair][:, qb * P : (qb + 1) * P], tpt[:, :]
                    )
                    emit_cblock_m(j, qb, act_only=True)

                fin = {}
                due = []
                for j, i in steps:
                    final_j = pair == 1 and j == NQ - 1
                    if i == 0:
                        # deadline: this j's QT/KT sweep must be fully emitted
                        ensure(("sw", j, pair))
                    if (pair, i) not in vp:
                        emit_transposes(pair, i, i + 1)
                    emit_S(j, i)
                    if i >= 4 * j:
                        qb = i - 4 * j
                        if final_j:
                            due.append(lambda j=j, qb=qb: final_tail(j, qb))
                        else:
                            due.append(lambda j=j, qb=qb: av_burst(j, qb))
                            if qb == 3:
                                due.append(lambda j=j: finalize(j))
                    drip(2.5 if (pair, i) <= (0, 2) else 1.1)
                    # AV bursts trail the exp they consume by ~2 tiles so
                    # the ACT pipeline keeps slack
                    while len(due) > (1 if final_j else 2):
                        due.pop(0)()
                    yield
                # non-yielding drain: pair 0's finalize(3) must be emitted
                # before pair 1's endgame cblocks read attn[3][0]
                while due:
                    due.pop(0)()

            def t_item(pair, i):
                return (0, ("t", pair, i), lambda: emit_transposes(pair, i, i + 1))

            def sweep_bg(ni, mi):
                # per-k-pair items are 3 DoubleRow matmuls of 512 cols at
                # 0.5 cyc/row (~320ns); evict is DVE
                return [
                    (1.5 if ix < NKD // 2 else 0, ("sw", ni, mi), it)
                    for w in (0, 1)
                    for ix, it in enumerate(sweep_items(ni, mi, w))
                ]

            # upfront: pair-0 ni=0 projection (DMA-paced) + first V transposes
            for w in (0, 1):
                for it in sweep_items(0, 0, w):
                    it()
            emit_transposes_pe(0, 0, 4)
            # bg order follows need-by and DMA-arrival order; pairs run
            # interleaved, so mi=0/mi=1 sweeps alternate per seq chunk
            bg.extend(sweep_bg(0, 1))
            bg.extend(t_item(1, i) for i in range(0, 4))
            bg.extend(sweep_bg(1, 0))
            bg.extend(sweep_bg(1, 1))
            bg.extend(t_item(0, i) for i in range(4, 8))
            bg.extend(t_item(1, i) for i in range(4, 8))
            bg.extend(sweep_bg(2, 0))
            bg.extend(sweep_bg(2, 1))
            bg.extend(t_item(0, i) for i in range(8, 12))
            bg.extend(t_item(1, i) for i in range(8, 12))
            bg.extend(sweep_bg(3, 0))
            bg.extend(sweep_bg(3, 1))
            bg.extend(t_item(0, i) for i in range(12, 16))
            bg.extend(t_item(1, i) for i in range(12, 16))
            g0, g1 = pair_gen(0), pair_gen(1)
            for _ in range(3):
                next(g0, None)
            alive = True
            while alive:
                alive = False
                for g in (g0, g1):
                    if next(g, "done") != "done":
                        alive = True
            while bg:
                drip(8)

    nc.compile()
    return nc


def make_in_maps(pos_encode_toks, Wq, bq, Wk, bk, W0, b0):
    x = np.asarray(pos_encode_toks, dtype=np.float32)
    Wq = np.asarray(Wq, dtype=np.float32)
    bq = np.asarray(bq, dtype=np.float32)
    Wk = np.asarray(Wk, dtype=np.float32)
    bk = np.asarray(bk, dtype=np.float32)
    W0 = np.asarray(W0, dtype=np.float32)
    in_maps = []
    fp8 = ml_dtypes.float8_e4m3

    def x_hi_lo(a):
        # x_hi plus unscaled residual (x_lo is within e4m3 normal range)
        hi = a.astype(fp8)
        lo = (a - hi.astype(np.float32)).astype(fp8)
        return hi, lo

    def w_hi_lo(a):
        # both shipped x16: W_lo would underflow e4m3 normals unscaled, and
        # 16*W_hi keeps every psum term on the same x16 scale (exact in fp8)
        hi = a.astype(fp8)
        hi16 = (hi.astype(np.float32) * 16.0).astype(fp8)
        lo = ((a - hi.astype(np.float32)) * 16.0).astype(fp8)
        return hi16, lo

    for core in range(8):
        b, g = divmod(core, 4)
        hs = slice(g * HD, (g + 1) * HD)
        xh, xl = x_hi_lo(np.ascontiguousarray(x[b].T))
        wqh, wql = w_hi_lo(np.ascontiguousarray(Wq[:, hs]))
        wkh, wkl = w_hi_lo(np.ascontiguousarray(Wk[:, hs]))
        in_maps.append(
            {
                "xT": xh,
                "xTl": xl,
                "Wq": wqh,
                "Wql": wql,
                "Wk": wkh,
                "Wkl": wkl,
                "bqt": np.ascontiguousarray(bq[hs].reshape(2, P).T),
                "bkt": np.ascontiguousarray(bk[hs].reshape(2, P).T),
                "W0": np.ascontiguousarray(W0[hs, :]).astype(ml_dtypes.bfloat16),
            }
        )
    return in_maps


def assemble(results, b0):
    out = np.zeros((2, S, D), dtype=np.float32)
    for core in range(8):
        b = core // 4
        out[b] += results[core]["out"].astype(np.float32)
    out += np.asarray(b0, dtype=np.float32)
    return out


def kernel(pos_encode_toks, Wq, bq, Wk, bk, W0, b0):
    from concourse.bass_utils import run_bass_kernel_spmd

    global _CACHED_NC
    if _CACHED_NC is None:
        _CACHED_NC = build_nc()
    in_maps = make_in_maps(pos_encode_toks, Wq, bq, Wk, bk, W0, b0)
    res = run_bass_kernel_spmd(_CACHED_NC, in_maps, core_ids=list(range(8)))
    return assemble(res.results, b0)



# revision 67
# speedup vs baseline: 1.1163x; 1.0259x over previous
"""Multi-head causal self-attention (V=Q variant) on 8 Trainium2 cores.

Sharding: batch (2) x head-group (4 groups of 4 heads). Each core computes
full-sequence attention for its 4 heads of one batch element, plus its slice
of the output projection; the host sums the 4 partial projections per batch
and adds b0.

Per core: xT [1024, 2048], Wq_s/Wk_s [1024, 256], W0_s [256, 1024].

Scores are computed transposed (S^T[kv, q]) so the softmax denominator falls
out of the AV matmul via a ones-column appended to V (V aliases Q in this
module -- the reference replicates that bug). The 1/sqrt(DK) scale is folded
into Wk/bk on the host. All matmul operands are bf16 (1 cycle/row on the PE);
PSUM accumulation stays fp32.

The two heads of a pair share one [128,1024] score PSUM tile so a single ACT
exp instruction covers both (halves ACT instruction count). V' tiles are
produced by DMA-engine transposes (XBAR) straight from the bf16 QT tiles,
keeping the PE out of the transpose path and DVE out of the evictions. ACT
(exp) is the attention-phase rate limiter, so program order interleaves
projection sub-sweeps and output-projection blocks into the PE's exp-wait
gaps via a drip queue. Causal diagonal tiles are narrowed to their valid
q-range (left-trimmed).
"""

import ml_dtypes
import numpy as np

import concourse.bacc as bacc
import concourse.mybir as mybir
from concourse.tile import TileContext

P = 128
S = 2048  # sequence length
D = 1024  # model dim
HD = 256  # head-group width (4 heads x 64)
DK = 64
NQ = 4  # q chunks of 512
NKV = 16  # kv chunks of 128
NKD = 8  # D chunks of 128
F32 = mybir.dt.float32
BF16 = mybir.dt.bfloat16
FP8 = mybir.dt.float8e4
DR = mybir.MatmulPerfMode.DoubleRow
EXP = mybir.ActivationFunctionType.Exp
SCALE = 1.0 / np.sqrt(DK)  # applied inside the exp activation

_CACHED_NC = None


def build_nc():
    nc = bacc.Bacc("TRN2", target_bir_lowering=False, debug=False, num_devices=8)
    # x2: host-swizzled to the SBUF layout [p, k, hi/lo, s]; w4: per-mi pack
    # of [16*Wq_hi, Wq_lo16, 16*Wk_hi, Wk_lo16] already in [p, 4, k, c] layout
    x2 = nc.declare_dram_parameter("x2", [P, NKD, 2, S], FP8, isOutput=False)
    w4_0 = nc.declare_dram_parameter("w4_0", [P, 4, NKD, P], FP8, isOutput=False)
    w4_1 = nc.declare_dram_parameter("w4_1", [P, 4, NKD, P], FP8, isOutput=False)
    bqt = nc.declare_dram_parameter("bqt", [P, 2], F32, isOutput=False)
    bkt = nc.declare_dram_parameter("bkt", [P, 2], F32, isOutput=False)
    W0 = nc.declare_dram_parameter("W0", [HD, D], BF16, isOutput=False)
    out = nc.declare_dram_parameter("out", [S, D], BF16, isOutput=True)

    with TileContext(nc) as tc:
        with (
            tc.tile_pool(name="const", bufs=1) as const,
            tc.tile_pool(name="xt", bufs=1) as xtp,
            tc.tile_pool(name="wqk", bufs=1) as wp,
            tc.tile_pool(name="vp", bufs=32) as vpool,
            tc.tile_pool(name="pt", bufs=34) as ptp,
            tc.tile_pool(name="dp", bufs=8) as dpool,
            tc.tile_pool(name="an", bufs=3) as anp,
            tc.tile_pool(name="ost", bufs=5) as ostp,
            tc.tile_pool(name="mm", bufs=4, space="PSUM") as mmp,
            tc.tile_pool(name="sps", bufs=2, space="PSUM") as spsum,
        ):
            ones_col = const.tile([P, 1], BF16)
            nc.gpsimd.memset(ones_col[:], 1.0)
            identity = const.tile([P, P], BF16)
            nc.gpsimd.memset(identity[:], 0.0)
            nc.gpsimd.affine_select(
                out=identity[:],
                in_=identity[:],
                compare_op=mybir.AluOpType.not_equal,
                fill=1.0,
                base=0,
                pattern=[[-1, P]],
                channel_multiplier=1,
            )
            # triangular mask [128,128] bf16: keep (1.0) where q >= kv
            tri = const.tile([P, P], BF16, name="tri")
            nc.gpsimd.memset(tri[:], 1.0)
            nc.gpsimd.affine_select(
                out=tri[:],
                in_=tri[:],
                compare_op=mybir.AluOpType.is_ge,
                fill=0.0,
                base=0,
                pattern=[[1, P]],
                channel_multiplier=-1,
            )
            # ACT exp-table warmup while DMAs run
            warm = const.tile([P, 8], F32, name="warm")
            nc.gpsimd.memset(warm[:], 0.0)
            nc.scalar.activation(out=warm[:], in_=warm[:], func=EXP)
            # PE p-state warmup: ~3us of dummy transposes on the identity
            # tile (no DMA dependency) so the PE is at full clock when the
            # first projection data lands
            pewarm = mmp.tile([P, P], BF16, name="ps")
            for _ in range(60):
                nc.tensor.transpose(pewarm[:], identity[:], identity[:])

            bq_sb = const.tile([P, 2], F32)
            bk_sb = const.tile([P, 2], F32)
            w0_sb = [const.tile([P, D], BF16, name=f"w0_{kc}") for kc in range(2)]
            # QT/KT as [mi][ni] tiles of [128, 512] for fine-grained deps
            QT = [
                [const.tile([P, 512], BF16, name=f"qt{mi}_{ni}") for ni in range(NQ)]
                for mi in range(2)
            ]
            KT = [
                [const.tile([P, 512], BF16, name=f"kt{mi}_{ni}") for ni in range(NQ)]
                for mi in range(2)
            ]
            # normalized attention (transposed), per q-chunk and head-pair
            attn = [
                [const.tile([P, 512], BF16, name=f"attn{j}_{p}") for p in range(2)]
                for j in range(4)
            ]

            # first k-pair of each weight as its own tile/DMA so the first
            # projection sub-sweep starts as soon as ~130KB has landed
            # DMA order = first-exp critical path: the mi=0 weight pack, the
            # first 512-col sliver of x (hi+lo, all 8 k-tiles, one DMA);
            # mi=1 pack and remaining x col-blocks follow.
            w4sb = [wp.tile([P, 4, NKD, P], FP8, name=f"w4_{m}") for m in range(2)]
            x2sb = xtp.tile([P, NKD, 2, S], FP8, name="x2")
            x2_view = x2
            nc.sync.dma_start(out=w4sb[0][:], in_=w4_0[:, :, :, :])
            nc.sync.dma_start(out=x2sb[:, :, :, 0:512], in_=x2_view[:, :, :, 0:512])
            nc.sync.dma_start(out=bq_sb[:], in_=bqt[:, :])
            nc.sync.dma_start(out=bk_sb[:], in_=bkt[:, :])
            nc.sync.dma_start(out=x2sb[:, :, :, 512:1024], in_=x2_view[:, :, :, 512:1024])
            nc.sync.dma_start(out=w4sb[1][:], in_=w4_1[:, :, :, :])
            nc.sync.dma_start(
                out=x2sb[:, :, :, 1024:2048], in_=x2_view[:, :, :, 1024:2048]
            )
            for kc in range(2):
                nc.sync.dma_start(out=w0_sb[kc][:], in_=W0[kc * P : (kc + 1) * P, :])

            def wslice(mi, idx, kp):
                return w4sb[mi][:, idx, 2 * kp : 2 * kp + 2, :]

            def sweep_items(ni, mi, which):
                """One projection (q or k) sub-sweep as emit-thunks.

                fp8 DoubleRow with residual compensation, all terms scaled
                x16 so they share ONE psum group (DVE reads a single PSUM
                stream): (16 W_hi).x_hi + (16 W_lo).x_hi + W_hi.(16 x_lo).
                The eviction computes psum/16 + bias in one tensor_scalar.
                """
                half, col = divmod(ni, 2)
                c0 = half * 1024 + col * 512
                ps = mmp.tile([P, 512], F32, name="ps")
                hi_idx, lo_idx = (0, 1) if which == 0 else (2, 3)
                bias = bq_sb if which == 0 else bk_sb
                dstT = QT if which == 0 else KT

                def xsl(kp, t):
                    return x2sb[:, 2 * kp : 2 * kp + 2, t, c0 : c0 + 512]

                def mk(kp):
                    def go():
                        nc.tensor.matmul(
                            ps[:],
                            lhsT=wslice(mi, hi_idx, kp),
                            rhs=xsl(kp, 0),
                            start=(kp == 0),
                            stop=False,
                            perf_mode=DR,
                        )
                        nc.tensor.matmul(
                            ps[:],
                            lhsT=wslice(mi, hi_idx, kp),
                            rhs=xsl(kp, 1),
                            start=False,
                            stop=False,
                            perf_mode=DR,
                        )
                        nc.tensor.matmul(
                            ps[:],
                            lhsT=wslice(mi, lo_idx, kp),
                            rhs=xsl(kp, 0),
                            start=False,
                            stop=(kp == NKD // 2 - 1),
                            perf_mode=DR,
                        )
                    return go

                def evict():
                    # the first sweeps evict on ACT, which is idle until the
                    # first exp; later sweeps use DVE
                    if ni == 0 and mi == 0:
                        nc.scalar.activation(
                            out=dstT[mi][ni][:, :],
                            in_=ps[:],
                            func=mybir.ActivationFunctionType.Identity,
                            bias=bias[:, mi : mi + 1],
                            scale=1.0 / 16.0,
                        )
                    else:
                        nc.vector.tensor_scalar(
                            out=dstT[mi][ni][:, :],
                            in0=ps[:],
                            scalar1=1.0 / 16.0,
                            scalar2=bias[:, mi : mi + 1],
                            op0=mybir.AluOpType.mult,
                            op1=mybir.AluOpType.add,
                        )

                return [mk(kp) for kp in range(NKD // 2)] + [evict]

            vp = {}

            def emit_transposes(pair, i_lo, i_hi):
                # V' tiles [128, 132] via XBAR dma transpose from QT:
                # A data 0:64, A one 64, B data 66:130, B one 130.
                for i in range(i_lo, i_hi):
                    if (pair, i) in vp:
                        continue
                    src = QT[pair][i // 4][:, (i % 4) * P : (i % 4 + 1) * P]
                    # B data sits at col 80: XBAR writes require the out
                    # offset to be 32-byte aligned (16 bf16 cols)
                    vt = vpool.tile([P, 148], BF16, name="vt")
                    nc.sync.dma_start_transpose(out=vt[:, 0:64], in_=src[0:64, :])
                    nc.sync.dma_start_transpose(out=vt[:, 80:144], in_=src[64:128, :])
                    nc.gpsimd.tensor_copy(vt[:, 64:65], ones_col[:])
                    nc.gpsimd.tensor_copy(vt[:, 144:145], ones_col[:])
                    vp[(pair, i)] = vt

            def emit_transposes_pe(pair, i_lo, i_hi):
                # startup variant: PE transposes (PE is idle while DMAs land,
                # and the DMA/HWDGE path is saturated with input loads)
                tp = mmp.tile([P, 512], BF16, name="ps")
                for i in range(i_lo, i_hi):
                    c = (i - i_lo) * P
                    nc.tensor.transpose(
                        tp[:, c : c + P],
                        QT[pair][i // 4][:, (i % 4) * P : (i % 4 + 1) * P],
                        identity[:],
                    )
                    vt = vpool.tile([P, 148], BF16, name="vt")
                    nc.vector.tensor_copy(vt[:, 0:64], tp[:, c : c + 64])
                    nc.vector.tensor_copy(vt[:, 80:144], tp[:, c + 64 : c + P])
                    nc.gpsimd.tensor_copy(vt[:, 64:65], ones_col[:])
                    nc.gpsimd.tensor_copy(vt[:, 144:145], ones_col[:])
                    vp[(pair, i)] = vt

            bg = []  # drip queue of (cost, key, thunk)
            late_cb = []  # endgame-deferred out-projection blocks
            carry = [0.0]

            def drip(budget):
                # carry-based pacing: costs are ~213ns PE units; fractional
                # budget accumulates so fill work spreads over the whole
                # attention phase instead of draining the queue early
                carry[0] += budget
                while bg and carry[0] >= bg[0][0]:
                    cost, _key, thunk = bg.pop(0)
                    thunk()
                    carry[0] -= cost

            def ensure(key):
                # need-driven drain: emit every queued item with this key now
                # (deadline reached), regardless of queue position
                rest, hits = [], []
                for item in bg:
                    (hits if item[1] == key else rest).append(item)
                if hits:
                    bg[:] = rest
                    for _cost, _key, thunk in hits:
                        thunk()

            cb_state = {}

            def emit_cblock_half(j, c, n, act_only=False):
                m = j * 4 + c
                if n == 0:
                    cb_state[(j, c)] = ostp.tile([P, D], BF16, name="ot")
                ot = cb_state[(j, c)]
                ps = mmp.tile([P, 512], F32, name="ps")
                for kc in range(2):
                    nc.tensor.matmul(
                        ps[:],
                        lhsT=attn[j][kc][:, c * P : (c + 1) * P],
                        rhs=w0_sb[kc][:, n * 512 : (n + 1) * 512],
                        start=(kc == 0),
                        stop=(kc == 1),
                    )
                osl = slice(n * 512, (n + 1) * 512)
                # steady state: DVE evicts (ACT is saturated with exp);
                # endgame (act_only): ACT evicts -- its exp stream is done
                # and the DVE queue is the endgame critical path
                if act_only:
                    nc.scalar.copy(ot[:, osl], ps[:])
                else:
                    nc.vector.tensor_copy(ot[:, osl], ps[:])
                if act_only:
                    # endgame: per-half DMA so the n=0 half transfers
                    # while the n=1 half is still evicting
                    nc.sync.dma_start(out=out[m * P : (m + 1) * P, osl], in_=ot[:, osl])
                elif n == 1:
                    nc.sync.dma_start(out=out[m * P : (m + 1) * P, :], in_=ot[:])

            def emit_cblock_m(j, c, act_only=False):
                for n in range(2):
                    emit_cblock_half(j, c, n, act_only)

            def pair_gen(pair):
                """Generator: one attention step per yield, so the two
                head-pairs' streams can interleave and fill each other's
                dependency stalls."""
                steps = [(j, i) for j in range(NQ) for i in range(4 * j + 4)]
                pabs = {}
                asbs = {}

                def emit_S(j, i):
                    off = max(0, i * P - j * 512)  # 0,128,256,384
                    w = 512 - off
                    kc = slice((i % 4) * P, (i % 4 + 1) * P)
                    sAB = spsum.tile([P, 1024], F32, name="sAB")
                    nc.tensor.matmul(
                        sAB[:, off:512],
                        lhsT=KT[pair][i // 4][0:64, kc],
                        rhs=QT[pair][j][0:64, off:512],
                    )
                    nc.tensor.matmul(
                        sAB[:, 512 + off : 1024],
                        lhsT=KT[pair][i // 4][64:128, kc],
                        rhs=QT[pair][j][64:128, off:512],
                    )
                    pAB = ptp.tile([P, 1024], BF16, name="pAB")
                    if w == 512:
                        nc.scalar.activation(
                            out=pAB[:, :], in_=sAB[:, :], func=EXP, scale=SCALE
                        )
                    else:
                        nc.scalar.activation(
                            out=pAB[:].rearrange("p (t c) -> p t c", t=2)[:, :, off:512],
                            in_=sAB[:].rearrange("p (t c) -> p t c", t=2)[:, :, off:512],
                            func=EXP,
                            scale=SCALE,
                        )
                    if i >= 4 * j:  # diagonal tile: mask the leading block
                        nc.vector.tensor_mul(
                            pAB[:, off : off + P], pAB[:, off : off + P], tri[:]
                        )
                        nc.vector.tensor_mul(
                            pAB[:, 512 + off : 512 + off + P],
                            pAB[:, 512 + off : 512 + off + P],
                            tri[:],
                        )
                    pabs[(j, i)] = pAB

                def av_burst(j, qb):
                    # transposed AV: out = attn-normal [128 q-part, 65] for
                    # one 128-q block -- the PE streams only 65 columns per
                    # kv tile (vs 512 in the [65, q] orientation) and the
                    # softmax denominator lands per-partition (column 64).
                    # One accumulation group per PSUM bank (full-bank slots;
                    # the PE tracks a single active accum range per bank).
                    for hh in range(2):
                        if qb == 0:
                            asbs[(j, hh)] = dpool.tile([P, 4, 65], F32, name="asb")
                        av = mmp.tile([P, 512], F32, name="ps")
                        for t in range(4 * j + qb + 1):
                            vt = vp[(pair, t)]
                            vsl = vt[:, 0:65] if hh == 0 else vt[:, 80:145]
                            nc.tensor.matmul(
                                av[:, 0:65],
                                lhsT=pabs[(j, t)][
                                    :, hh * 512 + qb * P : hh * 512 + (qb + 1) * P
                                ],
                                rhs=vsl,
                                start=(t == 0),
                                stop=(t == 4 * j + qb),
                            )
                        # prompt psum->sbuf eviction frees the bank slot
                        nc.vector.tensor_copy(asbs[(j, hh)][:, qb, :], av[:, 0:65])

                def finalize(j):
                    # per-qb Pool normalize_recip divides the 64 dk cols by
                    # col 64 (the ones-column row sum), writing bf16
                    # attn-normal; PE transposes assemble the attn^T tile
                    # the out-projection consumes.
                    an = anp.tile([P, 4, 2, 64], BF16, name="an")
                    tpt = mmp.tile([P, 4, P], BF16, name="ps")
                    for qb in range(4):
                        for hh in range(2):
                            nc.gpsimd.normalize_recip(
                                an[:, qb, hh, :],
                                asbs[(j, hh)][:, qb, 0:64],
                                asbs[(j, hh)][:, qb, 64:65],
                            )
                        # transpose [128 q, 2x64 hd] -> [128 hd, 128 q] on PE
                        nc.tensor.transpose(
                            tpt[:, qb, :], an[:, qb, :, :], identity[:]
                        )
                    nc.vector.tensor_copy(attn[j][pair][:, :], tpt[:, :, :])
                    if pair == 1:  # output projection becomes available
                        # j<=1 cblocks are deferred to the endgame stretch,
                        # where the PE otherwise idles at exp pace
                        for c in range(4):
                            for n in range(2):
                                bg.append(
                                    (
                                        2,
                                        ("cb", j, c),
                                        lambda j=j, c=c, n=n: emit_cblock_half(j, c, n),
                                    )
                                )

                def final_tail(j, qb):
                    # endgame (pair 1, last j): per-qb chain burst ->
                    # normalize -> transpose -> attn evict -> cblock, so the
                    # post-exp drain is pipelined across engines per q-block
                    av_burst(j, qb)
                    if qb == 0:
                        fin["an"] = anp.tile([P, 4, 2, 64], BF16, name="an")
                    an = fin["an"]
                    for hh in range(2):
                        nc.gpsimd.normalize_recip(
                            an[:, qb, hh, :],
                            asbs[(j, hh)][:, qb, 0:64],
                            asbs[(j, hh)][:, qb, 64:65],
                        )
                    # per-qb transpose psum: its pool slot must not persist
                    # across later burst allocations (4-slot rotation)
                    tpt = mmp.tile([P, P], BF16, name="ps")
                    nc.tensor.transpose(tpt[:, :], an[:, qb, :, :], identity[:])
                    nc.vector.tensor_copy(
                        attn[j][pair][:, qb * P : (qb + 1) * P], tpt[:, :]
                    )
                    emit_cblock_m(j, qb, act_only=True)

                fin = {}
                due = []
                for j, i in steps:
                    final_j = pair == 1 and j == NQ - 1
                    if i == 0:
                        # deadline: this j's QT/KT sweep must be fully emitted
                        ensure(("sw", j, pair))
                    if (pair, i) not in vp:
                        emit_transposes(pair, i, i + 1)
                    emit_S(j, i)
                    if i >= 4 * j:
                        qb = i - 4 * j
                        if final_j:
                            due.append(lambda j=j, qb=qb: final_tail(j, qb))
                        else:
                            due.append(lambda j=j, qb=qb: av_burst(j, qb))
                            if qb == 3:
                                due.append(lambda j=j: finalize(j))
                    drip(2.5 if (pair, i) <= (0, 2) else 1.1)
                    # AV bursts trail the exp they consume by ~2 tiles so
                    # the ACT pipeline keeps slack
                    while len(due) > (1 if final_j else 2):
                        due.pop(0)()
                    yield
                # non-yielding drain: pair 0's finalize(3) must be emitted
                # before pair 1's endgame cblocks read attn[3][0]
                while due:
                    due.pop(0)()

            def t_item(pair, i):
                return (0, ("t", pair, i), lambda: emit_transposes(pair, i, i + 1))

            def sweep_bg(ni, mi):
                # per-k-pair items are 3 DoubleRow matmuls of 512 cols at
                # 0.5 cyc/row (~320ns); evict is DVE
                return [
                    (1.5 if ix < NKD // 2 else 0, ("sw", ni, mi), it)
                    for w in (0, 1)
                    for ix, it in enumerate(sweep_items(ni, mi, w))
                ]

            # upfront: pair-0 ni=0 projection (DMA-paced) + first V transposes
            for w in (0, 1):
                for it in sweep_items(0, 0, w):
                    it()
            emit_transposes_pe(0, 0, 4)
            # bg order follows need-by and DMA-arrival order; pairs run
            # interleaved, so mi=0/mi=1 sweeps alternate per seq chunk
            bg.extend(sweep_bg(0, 1))
            bg.extend(t_item(1, i) for i in range(0, 4))
            bg.extend(sweep_bg(1, 0))
            bg.extend(sweep_bg(1, 1))
            bg.extend(t_item(0, i) for i in range(4, 8))
            bg.extend(t_item(1, i) for i in range(4, 8))
            bg.extend(sweep_bg(2, 0))
            bg.extend(sweep_bg(2, 1))
            bg.extend(t_item(0, i) for i in range(8, 12))
            bg.extend(t_item(1, i) for i in range(8, 12))
            bg.extend(sweep_bg(3, 0))
            bg.extend(sweep_bg(3, 1))
            bg.extend(t_item(0, i) for i in range(12, 16))
            bg.extend(t_item(1, i) for i in range(12, 16))
            g0, g1 = pair_gen(0), pair_gen(1)
            for _ in range(3):
                next(g0, None)
            alive = True
            while alive:
                alive = False
                for g in (g0, g1):
                    if next(g, "done") != "done":
                        alive = True
            while bg:
                drip(8)

    nc.compile()
    return nc


def make_in_maps(pos_encode_toks, Wq, bq, Wk, bk, W0, b0):
    x = np.asarray(pos_encode_toks, dtype=np.float32)
    Wq = np.asarray(Wq, dtype=np.float32)
    bq = np.asarray(bq, dtype=np.float32)
    Wk = np.asarray(Wk, dtype=np.float32)
    bk = np.asarray(bk, dtype=np.float32)
    W0 = np.asarray(W0, dtype=np.float32)
    in_maps = []
    fp8 = ml_dtypes.float8_e4m3

    def x_hi_lo(a):
        # x_hi plus unscaled residual (x_lo is within e4m3 normal range)
        hi = a.astype(fp8)
        lo = (a - hi.astype(np.float32)).astype(fp8)
        return hi, lo

    def w_hi_lo(a):
        # both shipped x16: W_lo would underflow e4m3 normals unscaled, and
        # 16*W_hi keeps every psum term on the same x16 scale (exact in fp8)
        hi = a.astype(fp8)
        hi16 = (hi.astype(np.float32) * 16.0).astype(fp8)
        lo = ((a - hi.astype(np.float32)) * 16.0).astype(fp8)
        return hi16, lo

    def wm_pack(a, mi):
        # [D, 128] slice -> [p, k, c]
        a128 = a[:, mi * P : (mi + 1) * P]
        return a128.reshape(NKD, P, P).transpose(1, 0, 2)

    for core in range(8):
        b, g = divmod(core, 4)
        hs = slice(g * HD, (g + 1) * HD)
        xh, xl = x_hi_lo(np.ascontiguousarray(x[b].T))
        wqh, wql = w_hi_lo(np.ascontiguousarray(Wq[:, hs]))
        wkh, wkl = w_hi_lo(np.ascontiguousarray(Wk[:, hs]))
        # [p, k, hi/lo, s] swizzle matching the SBUF tile layout
        x2 = np.ascontiguousarray(
            np.stack(
                [a.reshape(NKD, P, S).transpose(1, 0, 2) for a in (xh, xl)], axis=2
            )
        )
        w4 = [
            np.ascontiguousarray(
                np.stack([wm_pack(w, mi) for w in (wqh, wql, wkh, wkl)], axis=1)
            )
            for mi in range(2)
        ]
        in_maps.append(
            {
                "x2": x2,
                "w4_0": w4[0],
                "w4_1": w4[1],
                "bqt": np.ascontiguousarray(bq[hs].reshape(2, P).T),
                "bkt": np.ascontiguousarray(bk[hs].reshape(2, P).T),
                "W0": np.ascontiguousarray(W0[hs, :]).astype(ml_dtypes.bfloat16),
            }
        )
    return in_maps


def assemble(results, b0):
    out = np.zeros((2, S, D), dtype=np.float32)
    for core in range(8):
        b = core // 4
        out[b] += results[core]["out"].astype(np.float32)
    out += np.asarray(b0, dtype=np.float32)
    return out


def kernel(pos_encode_toks, Wq, bq, Wk, bk, W0, b0):
    from concourse.bass_utils import run_bass_kernel_spmd

    global _CACHED_NC
    if _CACHED_NC is None:
        _CACHED_NC = build_nc()
    in_maps = make_in_maps(pos_encode_toks, Wq, bq, Wk, bk, W0, b0)
    res = run_bass_kernel_spmd(_CACHED_NC, in_maps, core_ids=list(range(8)))
    return assemble(res.results, b0)

